# revision 1
# baseline (speedup 1.0000x reference)
"""Trainium2 Bass kernel v5 for nn_KineticModel (gnn_message_passing).

Math (from the reference):
    conc    = scatter(conc_balanced, exp(log_conc_unbalanced))      # [8192]
    logc    = log(conc)                                             # [8192]
    logv    = log_kcat + relu(-S).T @ logc                          # [16384]
    v       = exp(logv)
    dcdt    = (S @ v)[:7680]

Sharding: reaction axis across 8 cores (2048 reactions each).

v5 design:
  * S ships as fp8-e4m3 (entries in {-2..2} -> exact): 31 MB/core HBM,
    2 MB (s_a) / 960 KB (s_b) DMA transfers.  relu(-S) precomputed on
    the host for layout A; layout B is S^T over balanced species only.
  * Both matvecs keep the vector stationary as a bf16 [hi, lo] column
    pair (f32-grade accuracy via PSUM accumulation) and stream S (fp8
    moving operand) at N<=512.  Mixed bf16 x fp8 matmuls.
  * Column tiling: matvec1 packs 4 species blocks into PE column groups
    {0,32,64,96} (concurrent matmuls); matvec2 packs 2 reaction blocks
    into groups {0,32}.  The first matmul per PSUM bank uses a
    zero-padded M=98/M=34 stationary so has_written covers all group
    rows; later matmuls accumulate.
  * All engine access patterns start at partitions 0/32/64/96.  The
    final hi+lo pair sum runs as a K=2 ones-vector matmul on the PE.
  * Single [1, 7680] bf16 output (15 KB/core).
"""

import sys

if "/opt/trn_rl_repo" not in sys.path:
    sys.path.insert(0, "/opt/trn_rl_repo")

import numpy as np
import ml_dtypes

import concourse.bacc as bacc
import concourse.mybir as mybir
from concourse.tile import TileContext
from concourse.bass_utils import run_bass_kernel_spmd
from concourse.masks import make_identity

F32 = mybir.dt.float32
BF16 = mybir.dt.bfloat16
FP8 = mybir.dt.float8e4
FP8_NP = ml_dtypes.float8_e4m3

N_SPECIES = 8192
N_RXN = 16384
N_BAL = 7680
N_CORES = 8
R_CORE = N_RXN // N_CORES        # 2048 reactions per core
SB = N_SPECIES // 128            # 64 species blocks
RB = R_CORE // 128               # 16 reaction blocks per core
GA = 8                           # species blocks per s_a DMA (2 MB transfers)
NGA = SB // GA                   # 8 s_a mega-tiles
NQ = 16                          # species slices for matvec2
QS = N_BAL // NQ                 # 480 balanced species per slice

_CACHE = {}


def _build_nc(reps=1):
    nc = bacc.Bacc(None, target_bir_lowering=False, debug=False)
    s_a = nc.declare_dram_parameter("s_a", [NGA, 128, GA * R_CORE], FP8,
                                    isOutput=False)
    s_b = nc.declare_dram_parameter("s_b", [NQ, 128, RB * QS], FP8,
                                    isOutput=False)
    xa = nc.declare_dram_parameter("xa", [128, SB], F32, isOutput=False)
    xb = nc.declare_dram_parameter("xb", [128, SB], F32, isOutput=False)
    kcat = nc.declare_dram_parameter("kcat", [128, RB], F32, isOutput=False)
    out = nc.declare_dram_parameter("out", [1, N_BAL], BF16, isOutput=True)

    ts = mybir.AluOpType
    with TileContext(nc) as tc:
        with (
            tc.tile_pool(name="small", bufs=1) as small,
            tc.tile_pool(name="sa", bufs=3) as sa_pool,
            tc.tile_pool(name="sbp", bufs=3) as sb_pool,
            tc.tile_pool(name="stage", bufs=2) as stage_pool,
            tc.tile_pool(name="psv", bufs=1, space="PSUM") as psv_pool,
            tc.tile_pool(name="pvt", bufs=1, space="PSUM") as pvt_pool,
            tc.tile_pool(name="psd", bufs=2, space="PSUM") as psd_pool,
            tc.tile_pool(name="tps", bufs=1, space="PSUM") as tps_pool,
        ):
            ident = small.tile([2, 2], F32, tag="ident")
            make_identity(nc, ident)
            ones2 = small.tile([2, 1], BF16, tag="ones2")
            nc.gpsimd.memset(ones2, 1.0)
            for rep in range(reps):
                _body(nc, tc, ts, small, sa_pool, sb_pool, stage_pool,
                      psv_pool, pvt_pool, psd_pool, tps_pool, ident, ones2,
                      s_a, s_b, xa, xb, kcat, out)
    nc.compile()
    return nc


def _body(nc, tc, ts, small, sa_pool, sb_pool, stage_pool, psv_pool,
          pvt_pool, psd_pool, tps_pool, ident, ones2,
          s_a, s_b, xa, xb, kcat, out):
    # ---- logc = Ln(xa) + xb, split into interleaved hi/lo bf16 ----
    xa_t = small.tile([128, SB], F32, tag="xa")
    xb_t = small.tile([128, SB], F32, tag="xb")
    kcat_t = small.tile([128, RB], F32, tag="kcat")
    nc.sync.dma_start(out=xa_t, in_=xa[:])
    nc.sync.dma_start(out=xb_t, in_=xb[:])
    nc.sync.dma_start(out=kcat_t, in_=kcat[:])

    lg = small.tile([128, SB], F32, tag="lg")
    nc.scalar.activation(lg, xa_t, mybir.ActivationFunctionType.Ln)
    logc = small.tile([128, SB], F32, tag="logc")
    nc.vector.tensor_tensor(out=logc, in0=lg, in1=xb_t, op=ts.add)

    logc_hl = small.tile([128, 2 * SB], BF16, tag="logc_hl")
    nc.vector.tensor_copy(out=logc_hl[:, 0 : 2 * SB : 2], in_=logc)
    lh_f = small.tile([128, SB], F32, tag="lh_f")
    nc.vector.tensor_copy(out=lh_f, in_=logc_hl[:, 0 : 2 * SB : 2])
    nc.vector.tensor_tensor(
        out=logc_hl[:, 1 : 2 * SB : 2], in0=logc, in1=lh_f, op=ts.subtract
    )

    # Zero-padded stationary for the first matmul per PSUM bank: writing
    # rows 0..97 once sets has_written across all col-group rows, so the
    # later col-tiled matmuls accumulate cleanly (HW and CoreSim agree).
    lw98 = small.tile([128, 98], BF16, tag="lw98")
    nc.gpsimd.memset(lw98, 0.0)
    nc.vector.tensor_copy(out=lw98[:, 0:2], in_=logc_hl[:, 0:2])

    # ---- matvec1 (4-way col-tiled): psum_v[32j:32j+2] += logc.T @ A ----
    psum_v = psv_pool.tile([98, R_CORE], F32, tag="psum_v")
    for g in range(NGA):
        at = sa_pool.tile([128, GA * R_CORE], FP8, tag="sa")
        nc.sync.dma_start(out=at, in_=s_a[g])
        for b in range(GA):
            sb = g * GA + b
            j = b % 4
            for c in range(4):
                if sb == 0:
                    nc.tensor.matmul(
                        psum_v[:, c * 512 : (c + 1) * 512],
                        lw98,
                        at[:, c * 512 : (c + 1) * 512],
                        start=True,
                        stop=False,
                        skip_group_check=True,
                    )
                else:
                    nc.tensor.matmul(
                        psum_v[32 * j : 32 * j + 2, c * 512 : (c + 1) * 512],
                        logc_hl[:, 2 * sb : 2 * sb + 2],
                        at[:, b * R_CORE + c * 512 : b * R_CORE + (c + 1) * 512],
                        start=False,
                        stop=(sb == SB - 1),
                        skip_group_check=True,
                        tile_position=(0, 32 * j),
                    )

    # ---- fold 4 groups -> [2, 2048], transpose -> [128, 32], v path ----
    # (a DVE/ACT op may read at most one PSUM operand: evacuate row-pairs
    # to SBUF first, splitting the copies between the two engines)
    pva = small.tile([2, R_CORE], F32, tag="pva")
    nc.scalar.activation(pva, psum_v[0:2, :], mybir.ActivationFunctionType.Copy)
    pvb = small.tile([2, R_CORE], F32, tag="pvb")
    nc.vector.tensor_copy(out=pvb, in_=psum_v[32:34, :])
    pvc = small.tile([2, R_CORE], F32, tag="pvc")
    nc.scalar.activation(pvc, psum_v[64:66, :], mybir.ActivationFunctionType.Copy)
    pvd = small.tile([2, R_CORE], F32, tag="pvd")
    nc.vector.tensor_copy(out=pvd, in_=psum_v[96:98, :])
    f01 = small.tile([2, R_CORE], F32, tag="f01")
    nc.vector.tensor_tensor(out=f01, in0=pva, in1=pvb, op=ts.add)
    f23 = small.tile([2, R_CORE], F32, tag="f23")
    nc.vector.tensor_tensor(out=f23, in0=pvc, in1=pvd, op=ts.add)
    fall = small.tile([2, R_CORE], F32, tag="fall")
    nc.vector.tensor_tensor(out=fall, in0=f01, in1=f23, op=ts.add)

    pvT = pvt_pool.tile([128, 2 * RB], F32, tag="pvT")
    for rb in range(RB):
        nc.tensor.transpose(
            pvT[:, 2 * rb : 2 * rb + 2],
            fall[:, rb * 128 : (rb + 1) * 128],
            ident,
        )
    pvT_sb = small.tile([128, 2 * RB], F32, tag="pvT_sb")
    nc.vector.tensor_copy(out=pvT_sb, in_=pvT)
    lv = small.tile([128, RB], F32, tag="lv")
    nc.vector.tensor_tensor(
        out=lv, in0=pvT_sb[:, 0 : 2 * RB : 2], in1=pvT_sb[:, 1 : 2 * RB : 2],
        op=ts.add,
    )
    lvk = small.tile([128, RB], F32, tag="lvk")
    nc.vector.tensor_tensor(out=lvk, in0=lv, in1=kcat_t, op=ts.add)
    v_f = small.tile([128, RB], F32, tag="v_f")
    nc.scalar.activation(v_f, lvk, mybir.ActivationFunctionType.Exp)

    v_hl = small.tile([128, 2 * RB], BF16, tag="v_hl")
    nc.vector.tensor_copy(out=v_hl[:, 0 : 2 * RB : 2], in_=v_f)
    vh_f = small.tile([128, RB], F32, tag="vh_f")
    nc.vector.tensor_copy(out=vh_f, in_=v_hl[:, 0 : 2 * RB : 2])
    nc.vector.tensor_tensor(
        out=v_hl[:, 1 : 2 * RB : 2], in0=v_f, in1=vh_f, op=ts.subtract
    )

    vw34 = small.tile([128, 34], BF16, tag="vw34")
    nc.gpsimd.memset(vw34, 0.0)
    nc.vector.tensor_copy(out=vw34[:, 0:2], in_=v_hl[:, 0:2])

    # ---- matvec2 (2-way col-tiled): psum_dc[32j:32j+2] += v.T @ S^T ----
    ost = stage_pool.tile([1, N_BAL], BF16, tag="ost")
    for q in range(NQ):
        psum_dc = psd_pool.tile([34, QS], F32, tag="psum_dc")
        bt = sb_pool.tile([128, RB * QS], FP8, tag="sb")
        nc.sync.dma_start(out=bt, in_=s_b[q])
        for rb in range(RB):
            j = rb % 2
            if rb == 0:
                nc.tensor.matmul(
                    psum_dc,
                    vw34,
                    bt[:, 0:QS],
                    start=True,
                    stop=False,
                    skip_group_check=True,
                )
            else:
                nc.tensor.matmul(
                    psum_dc[32 * j : 32 * j + 2, :],
                    v_hl[:, 2 * rb : 2 * rb + 2],
                    bt[:, rb * QS : (rb + 1) * QS],
                    start=False,
                    stop=(rb == RB - 1),
                    skip_group_check=True,
                    tile_position=(0, 32 * j),
                )
        # evacuate both group row-pairs to SBUF (bf16), then sum all four
        # rows with two accumulating K=2 ones-vector matmuls (avoids
        # partition-1 engine APs and two-PSUM-operand DVE ops).
        ta = stage_pool.tile([2, QS], BF16, tag="ta")
        nc.scalar.activation(ta, psum_dc[0:2, :],
                             mybir.ActivationFunctionType.Copy)
        tb = stage_pool.tile([2, QS], BF16, tag="tb")
        nc.vector.tensor_copy(out=tb, in_=psum_dc[32:34, :])
        ps1 = tps_pool.tile([1, QS], F32, tag="ps1")
        nc.tensor.matmul(ps1, ones2, ta, start=True, stop=False,
                         skip_group_check=True)
        nc.tensor.matmul(ps1, ones2, tb, start=False, stop=True,
                         skip_group_check=True)
        nc.vector.tensor_copy(out=ost[:, q * QS : (q + 1) * QS], in_=ps1)
    nc.sync.dma_start(out=out[:], in_=ost)


def _prep_inputs(conc_balanced, S, balanced_species, unbalanced_species,
                 log_conc_unbalanced, log_kcat):
    """Host-side shard + layout prep (pure data movement / dtype casts)."""
    in_maps = []
    xa_full = np.ones(N_SPECIES, dtype=np.float32)
    xb_full = np.zeros(N_SPECIES, dtype=np.float32)
    xa_full[np.asarray(balanced_species)] = np.asarray(conc_balanced)
    xb_full[np.asarray(unbalanced_species)] = np.asarray(log_conc_unbalanced)
    xa_pm = np.ascontiguousarray(xa_full.reshape(SB, 128).T)
    xb_pm = np.ascontiguousarray(xb_full.reshape(SB, 128).T)

    S = np.asarray(S)
    log_kcat = np.asarray(log_kcat)
    for c in range(N_CORES):
        r0 = c * R_CORE
        sl = S[:, r0 : r0 + R_CORE]                    # [8192, 2048] f32
        # s_a[g, p, b*2048 + r] = relu(-S[(g*GA+b)*128 + p, r0 + r])
        s_a = np.ascontiguousarray(
            np.maximum(-sl, 0.0).astype(FP8_NP)
            .reshape(NGA, GA, 128, R_CORE)
            .transpose(0, 2, 1, 3)
            .reshape(NGA, 128, GA * R_CORE)
        )
        # s_b[q, p, rb*QS + s] = S[q*QS + s, r0 + rb*128 + p]
        s_b = np.ascontiguousarray(
            sl[:N_BAL].astype(FP8_NP)                  # [7680, 2048]
            .reshape(NQ, QS, RB, 128)
            .transpose(0, 3, 2, 1)
            .reshape(NQ, 128, RB * QS)
        )
        kcat_pm = np.ascontiguousarray(
            log_kcat[r0 : r0 + R_CORE].astype(np.float32).reshape(RB, 128).T
        )
        in_maps.append(
            {"s_a": s_a, "s_b": s_b, "xa": xa_pm, "xb": xb_pm,
             "kcat": kcat_pm}
        )
    return in_maps


def kernel(**inputs) -> np.ndarray:
    if "nc" not in _CACHE:
        _CACHE["nc"] = _build_nc()
    nc = _CACHE["nc"]
    in_maps = _prep_inputs(**inputs)
    res = run_bass_kernel_spmd(nc, in_maps, core_ids=list(range(N_CORES)))
    acc = np.zeros(N_BAL, dtype=np.float64)
    for c in range(N_CORES):
        acc += res.results[c]["out"][0].astype(np.float64)
    return acc.astype(np.float32)

